# revision 18
# baseline (speedup 1.0000x reference)
"""GCNConv (COO SpMM + feature transform) distributed over 8 NeuronCores.

out = segment_sum(x[cols] * vals, rows) @ weight

Gather-free design. The hardware on this image has no fast indirect gather
(SWDGE indirect DMA costs ~1.4us per 128 gathered rows; the extended-ucode
dma_gather/ap_gather paths are Q7-software-rate bound at ~28ns per gathered
column), so instead of gathering x rows per edge, each core streams ALL of x
once in a host-chosen static layout and performs the gather+segment-sum as
dense one-hot matmuls:

 - Destination rows are split into 8 blocks of 12500; core k owns the edges
   targeting its rows (edges arrive sorted by destination row).
 - Prologue (on device, interleaved two batches ahead of the main loop so
   the PE stream pipelines): xW = x @ weight per 64-node source block from a
   feature-major staging of x (x_feat[f, n] = x[n, f]); laid down in SBUF as
   xres[p, b*32:(b+1)*32] = xW[64*b + p] (bf16, 64 partitions, resident).
   Applying W first is exact: W distributes over the segment sum.
 - Host groups core-k edges by (source block b, dest row r): each distinct
   pair is one "fragment" column m; bval[p, bvoff_b + m] = sum of vals of
   edges (col = 64*b + p  ->  r), bf16 (fp8-e4m3 was measured at 2.4e-2
   total error, over the 2e-2 gate; e3m4 is unsupported by the PE here).
   64-node blocks keep the one-hot slab at ~128B/edge of DMA traffic.  One
   matmul per block
       fragT[32, W] = xres_b[64, 32].T @ bval_b[64, W]
   computes all of block b's contributions; 3 consecutive blocks stack per
   PSUM tile on the partition axis (PE out base partition is limited to
   0/32/64) with a per-tile column width W_t = max fragment count of its 3
   blocks across all 8 cores (rounded to 16) — the schedule is data-driven
   and the program is compiled per input shape (compile is host-side and
   untimed).  PSUM is cast to bf16 (alternating DVE/Activation) and stored.
 - Host adds the ~16 fragments per destination row (vectorized reduceat) —
   the same un-permute/merge step the harness contract already requires for
   assembling the full output from per-core results.

DMA descriptor generation is ~700ns per dma_start on the sync/activation
sequencers, so loads/stores are batched 24 blocks (8 PSUM tiles) at a time
and spread across the sync (loads) and activation (stores) queues.  Per
core the device moves ~60MB of plain sequential DMA and runs ~3.2k matmuls;
there is no GpSimd work at all.
"""

import os
import sys
import tempfile
import types

import numpy as np
import ml_dtypes

# A transiently-wedged device can leave a poisoned NEFF in the shared neuron
# compile cache, making every later invocation with the same cache key crash.
# Compiling is only a few seconds here, so use a fresh per-process cache.
os.environ["NEURON_COMPILE_CACHE_URL"] = tempfile.mkdtemp(prefix="neuron-cc-cache-")


def _install_ntff_hook_shim():
    """bass_utils' axon trace path imports antenv.axon_hooks, which this
    container image lacks.  Provide it (with the real ctypes-based profiler
    hook when available) so BASS_TRACE=1 in the environment doesn't crash."""
    if "antenv.axon_hooks" in sys.modules:
        return
    mod = types.ModuleType("antenv.axon_hooks")
    _h = [None]
    mod.set_axon_ntff_profile_hook = lambda h: _h.__setitem__(0, h)
    mod.get_axon_ntff_profile_hook = lambda: _h[0]
    sys.modules["antenv.axon_hooks"] = mod
    try:
        from trn_agent_boot.trn_boot import _ntff_profile_via_ctypes

        mod.set_axon_ntff_profile_hook(
            _ntff_profile_via_ctypes("/opt/axon/libaxon_pjrt.so")
        )
    except Exception:
        pass


_install_ntff_hook_shim()

import concourse.bass as bass
import concourse.mybir as mybir
import concourse.tile as tile
from concourse import bacc
from concourse.bass_utils import run_bass_kernel_spmd

N_NODES = 100_000
N_CORES = 8
RPC = N_NODES // N_CORES  # dest rows per core
F = 32
P = 128
BN = 64  # nodes per source block
GPI = 3  # blocks stacked per PSUM tile (PE out base partition: 0/32/64)
QPH = 8  # PSUM tiles per load/store batch
BPH = GPI * QPH  # blocks per batch (24)
NBLK = 1584  # source blocks of 64 nodes (100000 -> 1562.5, padded to 24|NBLK)
NTILE = NBLK // GPI
NHALF = NBLK // BPH  # batches (66)
PGB = 12  # blocks per prologue PSUM group (2 groups per batch)
XCH = 144  # blocks per x_feat chunk load (6 batches; fewer, bigger DMAs)

f32 = mybir.dt.float32
bf16 = mybir.dt.bfloat16

_compiled_cache = {}


def _build_program(wts):
    """wts: per-PSUM-tile fragment column widths (len NTILE, multiples of
    16).  The bval/frag layouts use the corresponding running offsets."""
    wts = list(wts)
    toff = np.concatenate([[0], np.cumsum(wts)])  # frag column offsets
    bvoff = np.concatenate(
        [[0], np.cumsum([wts[t // GPI] for t in range(NBLK)])]
    )  # per-block bval column offsets (block b gets width of its tile)
    nc = bacc.Bacc("TRN2", target_bir_lowering=False, debug=False)
    x_feat = nc.dram_tensor("x", [F, NBLK * BN], bf16, kind="ExternalInput")
    bval = nc.dram_tensor(
        "bval", [BN, int(bvoff[-1])], bf16, kind="ExternalInput"
    )
    w = nc.dram_tensor("w", [F, F], bf16, kind="ExternalInput")
    frag = nc.dram_tensor("frag", [96, int(toff[-1])], bf16, kind="ExternalOutput")

    with tile.TileContext(nc) as tc:
        with (
            tc.tile_pool(name="const", bufs=1) as cpool,
            tc.tile_pool(name="xf", bufs=3) as xfpool,
            tc.tile_pool(name="bv", bufs=4) as bvpool,
            tc.tile_pool(name="zf", bufs=4) as zfpool,
            tc.tile_pool(name="xw", bufs=2, space="PSUM") as xwpool,
            tc.tile_pool(name="ps", bufs=6, space="PSUM") as pspool,
        ):
            wt = cpool.tile([F, F], bf16)
            nc.sync.dma_start(wt[:], w[:])
            xres = cpool.tile([BN, NBLK * F], bf16)
            xft_tiles = {}

            def prologue(h):
                # xres[p, b*F:(b+1)*F] = (x @ W)[64b + p] for batch h's blocks
                b0 = h * BPH
                ch = b0 // XCH
                if b0 % XCH == 0:
                    t = xfpool.tile([F, XCH * BN], bf16, tag="xf")
                    nc.sync.dma_start(
                        t[:], x_feat[:, ch * XCH * BN : (ch + 1) * XCH * BN]
                    )
                    xft_tiles[ch] = t
                xft = xft_tiles[ch]
                xo = (b0 % XCH) * BN
                for gp in range(BPH // PGB):
                    xw = xwpool.tile([BN, PGB * F], f32, tag="xw")
                    for i in range(PGB):
                        nc.tensor.matmul(
                            out=xw[:, i * F : (i + 1) * F],
                            lhsT=xft[
                                :,
                                xo + (gp * PGB + i) * BN : xo
                                + (gp * PGB + i + 1) * BN,
                            ],
                            rhs=wt[:],
                            start=True,
                            stop=True,
                        )
                    blk0 = b0 + gp * PGB
                    nc.scalar.copy(
                        xres[:, blk0 * F : (blk0 + PGB) * F], xw[:]
                    )

            def mainloop(h):
                # fragments = xres_b.T @ bval_b; 3 blocks per PSUM tile
                t0 = h * QPH
                bvt = bvpool.tile(
                    [BN, int(bvoff[(h + 1) * BPH] - bvoff[h * BPH])],
                    bf16,
                    tag="bv",
                )
                nc.sync.dma_start(
                    bvt[:], bval[:, int(bvoff[h * BPH]) : int(bvoff[(h + 1) * BPH])]
                )
                bvb = int(bvoff[h * BPH])
                zf = zfpool.tile(
                    [96, int(toff[t0 + QPH] - toff[t0])], bf16, tag="zf"
                )
                for q in range(QPH):
                    t = t0 + q
                    wt_t = wts[t]
                    ps = pspool.tile([P, wt_t], f32, tag="ps")
                    for g in range(GPI):
                        blk = t * GPI + g
                        co = int(bvoff[blk]) - bvb
                        nc.tensor.matmul(
                            out=ps[g * F : (g + 1) * F, :],
                            lhsT=xres[:, blk * F : (blk + 1) * F],
                            rhs=bvt[:, co : co + wt_t],
                            start=True,
                            stop=True,
                        )
                    zo = int(toff[t] - toff[t0])
                    dst = zf[:, zo : zo + wt_t]
                    if q % 2 == 0:
                        nc.vector.tensor_copy(dst, ps[0:96, :])
                    else:
                        nc.scalar.copy(dst, ps[0:96, :])
                nc.sync.dma_start(
                    frag[:, int(toff[t0]) : int(toff[t0 + QPH])], zf[:]
                )

            # prologue runs two batches ahead so PE/DMA/copy streams pipeline
            for h in range(NHALF + 3):
                if h < NHALF:
                    prologue(h)
                if h >= 3:
                    mainloop(h - 3)
    nc.compile()
    return nc


def _prep_core(rows, cols, vals, k):
    """Sort core k's edges by (source block, dest row); identify fragments
    (distinct pairs). Returns per-edge and per-fragment index arrays."""
    lo = np.searchsorted(rows, k * RPC)
    hi = np.searchsorted(rows, (k + 1) * RPC)
    c = np.asarray(cols[lo:hi], dtype=np.int64)
    r = np.asarray(rows[lo:hi], dtype=np.int64) - k * RPC
    v = np.asarray(vals[lo:hi], dtype=np.float32)
    b = c >> 6
    p = c & 63
    order = np.lexsort((r, b))
    bs, rs, ps_, vs = b[order], r[order], p[order], v[order]
    if len(bs) == 0:
        z = np.zeros(0, np.int64)
        return (z, np.zeros(0, np.float32), z, z, z, z, np.zeros(NBLK, np.int64))
    newpair = np.r_[True, (bs[1:] != bs[:-1]) | (rs[1:] != rs[:-1])]
    pairidx = np.cumsum(newpair) - 1  # fragment id per edge
    starts = np.flatnonzero(newpair)
    fb = bs[starts]  # fragment source block
    fr = rs[starts]  # fragment dest row (core-local)
    m_k = np.bincount(fb, minlength=NBLK)
    firstfrag = np.r_[0, np.cumsum(m_k)[:-1]]
    fm = np.arange(len(fb)) - firstfrag[fb]  # within-block fragment index
    return ps_, vs, pairidx, fb, fr, fm, m_k


def _build_inputs(x, rows, cols, vals, weight):
    """Host prep: returns (wts, in_maps, metas)."""
    x = np.asarray(x, dtype=np.float32)
    weight = np.asarray(weight, dtype=np.float32)

    preps = [_prep_core(rows, cols, vals, k) for k in range(N_CORES)]
    m_all = np.stack([pr[6] for pr in preps])  # [cores, NBLK]
    m_tile = m_all.reshape(N_CORES, NTILE, GPI).max(axis=(0, 2))
    wts = np.maximum(((m_tile + 7) // 8) * 8, 16).astype(np.int64)
    assert wts.max() <= 512, f"fragment tile width {wts.max()} exceeds PSUM bank"
    toff = np.concatenate([[0], np.cumsum(wts)])
    bvw = wts[np.arange(NBLK) // GPI]
    bvoff = np.concatenate([[0], np.cumsum(bvw)])

    xp = np.zeros((NBLK * BN, F), np.float32)
    xp[:N_NODES] = x
    x_feat = np.ascontiguousarray(xp.T).astype(ml_dtypes.bfloat16)
    w_bf = weight.astype(ml_dtypes.bfloat16)

    in_maps = []
    metas = []
    for k in range(N_CORES):
        ps_, vs, pairidx, fb, fr, fm, m_k = preps[k]
        slab = np.zeros((BN, int(bvoff[-1])), np.float32)
        edge_col = (bvoff[fb] + fm)[pairidx]
        np.add.at(slab, (ps_, edge_col), vs)
        in_maps.append(
            {
                "x": x_feat,
                "bval": slab.astype(ml_dtypes.bfloat16),
                "w": w_bf,
            }
        )
        metas.append((fb, fr, fm))
    return tuple(wts.tolist()), toff, in_maps, metas


def kernel(x, rows, cols, vals, weight):
    wts, toff, in_maps, metas = _build_inputs(x, rows, cols, vals, weight)

    if wts not in _compiled_cache:
        _compiled_cache[wts] = _build_program(wts)
    nc = _compiled_cache[wts]

    res = run_bass_kernel_spmd(nc, in_maps, list(range(N_CORES)))

    out_full = np.zeros((N_NODES, F), np.float32)
    for k in range(N_CORES):
        fb, fr, fm = metas[k]
        # fragment (b, m): partitions [32*(b%GPI), +32), column toff[b//GPI]+m
        dv = (
            np.asarray(res.results[k]["frag"])
            .reshape(3, F, int(toff[-1]))
            .astype(np.float32)
        )
        fvals = dv[fb % GPI, :, toff[fb // GPI] + fm]  # [n_frag, F]
        order = np.argsort(fr, kind="stable")
        sv = fvals[order]
        sr = fr[order]
        seg = np.r_[True, sr[1:] != sr[:-1]]
        segstarts = np.flatnonzero(seg)
        out_full[k * RPC + sr[segstarts]] = np.add.reduceat(
            sv, segstarts, axis=0
        )
    return out_full


# revision 19
# speedup vs baseline: 1.3462x; 1.3462x over previous
"""GCNConv (COO SpMM + feature transform) distributed over 8 NeuronCores.

out = segment_sum(x[cols] * vals, rows) @ weight

Gather-free design. The hardware on this image has no fast indirect gather
(SWDGE indirect DMA costs ~1.4us per 128 gathered rows; the extended-ucode
dma_gather/ap_gather paths are Q7-software-rate bound at ~28ns per gathered
column), so instead of gathering x rows per edge, each core streams ALL of x
once in a host-chosen static layout and performs the gather+segment-sum as
dense one-hot matmuls:

 - Destination rows are split into 8 blocks of 12500; core k owns the edges
   targeting its rows (edges arrive sorted by destination row).
 - Prologue (on device, interleaved two batches ahead of the main loop so
   the PE stream pipelines): xW = x @ weight per 64-node source block from a
   feature-major staging of x (x_feat[f, n] = x[n, f]); laid down in SBUF as
   xres[p, b*32:(b+1)*32] = xW[64*b + p] (bf16, 64 partitions, resident).
   Applying W first is exact: W distributes over the segment sum.
 - Host groups core-k edges by (source block b, dest row r): each distinct
   pair is one "fragment" column m; bval[p, bvoff_b + m] = sum of vals of
   edges (col = 64*b + p  ->  r), bf16 (fp8-e4m3 was measured at 2.4e-2
   total error, over the 2e-2 gate; e3m4 is unsupported by the PE here).
   64-node blocks keep the one-hot slab at ~128B/edge of DMA traffic.  One
   matmul per block
       fragT[32, W] = xres_b[64, 32].T @ bval_b[64, W]
   computes all of block b's contributions; 3 consecutive blocks stack per
   PSUM tile on the partition axis (PE out base partition is limited to
   0/32/64) with a per-tile column width W_t = max fragment count of its 3
   blocks across all 8 cores (rounded to 16) — the schedule is data-driven
   and the program is compiled per input shape (compile is host-side and
   untimed).  PSUM is cast to bf16 (alternating DVE/Activation) and stored.
 - Host adds the ~16 fragments per destination row (vectorized reduceat) —
   the same un-permute/merge step the harness contract already requires for
   assembling the full output from per-core results.

DMA descriptor generation is ~700ns per dma_start on the sync/activation
sequencers, so loads/stores are batched 24 blocks (8 PSUM tiles) at a time
and spread across the sync (loads) and activation (stores) queues.  Per
core the device moves ~60MB of plain sequential DMA and runs ~3.2k matmuls;
there is no GpSimd work at all.
"""

import os
import sys
import tempfile
import types

import numpy as np
import ml_dtypes

# A transiently-wedged device can leave a poisoned NEFF in the shared neuron
# compile cache, making every later invocation with the same cache key crash.
# Compiling is only a few seconds here, so use a fresh per-process cache.
os.environ["NEURON_COMPILE_CACHE_URL"] = tempfile.mkdtemp(prefix="neuron-cc-cache-")


def _install_ntff_hook_shim():
    """bass_utils' axon trace path imports antenv.axon_hooks, which this
    container image lacks.  Provide it (with the real ctypes-based profiler
    hook when available) so BASS_TRACE=1 in the environment doesn't crash."""
    if "antenv.axon_hooks" in sys.modules:
        return
    mod = types.ModuleType("antenv.axon_hooks")
    _h = [None]
    mod.set_axon_ntff_profile_hook = lambda h: _h.__setitem__(0, h)
    mod.get_axon_ntff_profile_hook = lambda: _h[0]
    sys.modules["antenv.axon_hooks"] = mod
    try:
        from trn_agent_boot.trn_boot import _ntff_profile_via_ctypes

        mod.set_axon_ntff_profile_hook(
            _ntff_profile_via_ctypes("/opt/axon/libaxon_pjrt.so")
        )
    except Exception:
        pass


_install_ntff_hook_shim()

import concourse.bass as bass
import concourse.mybir as mybir
import concourse.tile as tile
from concourse import bacc
from concourse.bass_utils import run_bass_kernel_spmd

N_NODES = 100_000
N_CORES = 8
RPC = N_NODES // N_CORES  # dest rows per core
F = 32
P = 128
BN = 64  # nodes per source block
GPI = 3  # blocks stacked per PSUM tile (PE out base partition: 0/32/64)
QPH = 8  # PSUM tiles per load/store batch
BPH = GPI * QPH  # blocks per batch (24)
NBLK = 1584  # source blocks of 64 nodes (100000 -> 1562.5, padded to 24|NBLK)
NTILE = NBLK // GPI
NHALF = NBLK // BPH  # batches (66)
PGB = 12  # blocks per prologue PSUM group (2 groups per batch)
XCH = 144  # blocks per x_feat chunk load (6 batches; fewer, bigger DMAs)

f32 = mybir.dt.float32
bf16 = mybir.dt.bfloat16

_compiled_cache = {}


def _build_program(wts):
    """wts: per-PSUM-tile fragment column widths (len NTILE, multiples of
    16).  The bval/frag layouts use the corresponding running offsets."""
    wts = list(wts)
    toff = np.concatenate([[0], np.cumsum(wts)])  # frag column offsets
    bvoff = np.concatenate(
        [[0], np.cumsum([wts[t // GPI] for t in range(NBLK)])]
    )  # per-block bval column offsets (block b gets width of its tile)
    nc = bacc.Bacc("TRN2", target_bir_lowering=False, debug=False)
    x_feat = nc.dram_tensor("x", [F, NBLK * BN], bf16, kind="ExternalInput")
    bval = nc.dram_tensor(
        "bval", [BN, int(bvoff[-1])], bf16, kind="ExternalInput"
    )
    w = nc.dram_tensor("w", [F, F], bf16, kind="ExternalInput")
    frag = nc.dram_tensor("frag", [96, int(toff[-1])], bf16, kind="ExternalOutput")

    with tile.TileContext(nc) as tc:
        with (
            tc.tile_pool(name="const", bufs=1) as cpool,
            tc.tile_pool(name="xf", bufs=3) as xfpool,
            tc.tile_pool(name="bv", bufs=4) as bvpool,
            tc.tile_pool(name="zf", bufs=4) as zfpool,
            tc.tile_pool(name="xw", bufs=2, space="PSUM") as xwpool,
            tc.tile_pool(name="ps", bufs=6, space="PSUM") as pspool,
        ):
            wt = cpool.tile([F, F], bf16)
            nc.sync.dma_start(wt[:], w[:])
            xres = cpool.tile([BN, NBLK * F], bf16)
            xft_tiles = {}

            def prologue(h):
                # xres[p, b*F:(b+1)*F] = (x @ W)[64b + p] for batch h's blocks
                b0 = h * BPH
                ch = b0 // XCH
                if b0 % XCH == 0:
                    t = xfpool.tile([F, XCH * BN], bf16, tag="xf")
                    nc.sync.dma_start(
                        t[:], x_feat[:, ch * XCH * BN : (ch + 1) * XCH * BN]
                    )
                    xft_tiles[ch] = t
                xft = xft_tiles[ch]
                xo = (b0 % XCH) * BN
                for gp in range(BPH // PGB):
                    xw = xwpool.tile([BN, PGB * F], f32, tag="xw")
                    for i in range(PGB):
                        nc.tensor.matmul(
                            out=xw[:, i * F : (i + 1) * F],
                            lhsT=xft[
                                :,
                                xo + (gp * PGB + i) * BN : xo
                                + (gp * PGB + i + 1) * BN,
                            ],
                            rhs=wt[:],
                            start=True,
                            stop=True,
                        )
                    blk0 = b0 + gp * PGB
                    nc.vector.tensor_copy(
                        xres[:, blk0 * F : (blk0 + PGB) * F], xw[:]
                    )

            def mainloop(h):
                # fragments = xres_b.T @ bval_b; 3 blocks per PSUM tile
                t0 = h * QPH
                bvt = bvpool.tile(
                    [BN, int(bvoff[(h + 1) * BPH] - bvoff[h * BPH])],
                    bf16,
                    tag="bv",
                )
                nc.sync.dma_start(
                    bvt[:], bval[:, int(bvoff[h * BPH]) : int(bvoff[(h + 1) * BPH])]
                )
                bvb = int(bvoff[h * BPH])
                zf = zfpool.tile(
                    [96, int(toff[t0 + QPH] - toff[t0])], bf16, tag="zf"
                )
                for q in range(QPH):
                    t = t0 + q
                    wt_t = wts[t]
                    ps = pspool.tile([P, wt_t], f32, tag="ps")
                    for g in range(GPI):
                        blk = t * GPI + g
                        co = int(bvoff[blk]) - bvb
                        nc.tensor.matmul(
                            out=ps[g * F : (g + 1) * F, :],
                            lhsT=xres[:, blk * F : (blk + 1) * F],
                            rhs=bvt[:, co : co + wt_t],
                            start=True,
                            stop=True,
                        )
                    zo = int(toff[t] - toff[t0])
                    dst = zf[:, zo : zo + wt_t]
                    if q % 2 == 0:
                        nc.vector.tensor_copy(dst, ps[0:96, :])
                    else:
                        nc.scalar.copy(dst, ps[0:96, :])
                nc.scalar.dma_start(
                    frag[:, int(toff[t0]) : int(toff[t0 + QPH])], zf[:]
                )

            # prologue runs two batches ahead so PE/DMA/copy streams pipeline
            for h in range(NHALF + 3):
                if h < NHALF:
                    prologue(h)
                if h >= 3:
                    mainloop(h - 3)
    nc.compile()
    return nc


def _prep_core(rows, cols, vals, k):
    """Sort core k's edges by (source block, dest row); identify fragments
    (distinct pairs). Returns per-edge and per-fragment index arrays."""
    lo = np.searchsorted(rows, k * RPC)
    hi = np.searchsorted(rows, (k + 1) * RPC)
    c = np.asarray(cols[lo:hi], dtype=np.int64)
    r = np.asarray(rows[lo:hi], dtype=np.int64) - k * RPC
    v = np.asarray(vals[lo:hi], dtype=np.float32)
    b = c >> 6
    p = c & 63
    order = np.lexsort((r, b))
    bs, rs, ps_, vs = b[order], r[order], p[order], v[order]
    if len(bs) == 0:
        z = np.zeros(0, np.int64)
        return (z, np.zeros(0, np.float32), z, z, z, z, np.zeros(NBLK, np.int64))
    newpair = np.r_[True, (bs[1:] != bs[:-1]) | (rs[1:] != rs[:-1])]
    pairidx = np.cumsum(newpair) - 1  # fragment id per edge
    starts = np.flatnonzero(newpair)
    fb = bs[starts]  # fragment source block
    fr = rs[starts]  # fragment dest row (core-local)
    m_k = np.bincount(fb, minlength=NBLK)
    firstfrag = np.r_[0, np.cumsum(m_k)[:-1]]
    fm = np.arange(len(fb)) - firstfrag[fb]  # within-block fragment index
    return ps_, vs, pairidx, fb, fr, fm, m_k


def _build_inputs(x, rows, cols, vals, weight):
    """Host prep: returns (wts, in_maps, metas)."""
    x = np.asarray(x, dtype=np.float32)
    weight = np.asarray(weight, dtype=np.float32)

    preps = [_prep_core(rows, cols, vals, k) for k in range(N_CORES)]
    m_all = np.stack([pr[6] for pr in preps])  # [cores, NBLK]
    m_tile = m_all.reshape(N_CORES, NTILE, GPI).max(axis=(0, 2))
    wts = np.maximum(((m_tile + 7) // 8) * 8, 16).astype(np.int64)
    assert wts.max() <= 512, f"fragment tile width {wts.max()} exceeds PSUM bank"
    toff = np.concatenate([[0], np.cumsum(wts)])
    bvw = wts[np.arange(NBLK) // GPI]
    bvoff = np.concatenate([[0], np.cumsum(bvw)])

    xp = np.zeros((NBLK * BN, F), np.float32)
    xp[:N_NODES] = x
    x_feat = np.ascontiguousarray(xp.T).astype(ml_dtypes.bfloat16)
    w_bf = weight.astype(ml_dtypes.bfloat16)

    in_maps = []
    metas = []
    for k in range(N_CORES):
        ps_, vs, pairidx, fb, fr, fm, m_k = preps[k]
        slab = np.zeros((BN, int(bvoff[-1])), np.float32)
        edge_col = (bvoff[fb] + fm)[pairidx]
        np.add.at(slab, (ps_, edge_col), vs)
        in_maps.append(
            {
                "x": x_feat,
                "bval": slab.astype(ml_dtypes.bfloat16),
                "w": w_bf,
            }
        )
        metas.append((fb, fr, fm))
    return tuple(wts.tolist()), toff, in_maps, metas


def kernel(x, rows, cols, vals, weight):
    wts, toff, in_maps, metas = _build_inputs(x, rows, cols, vals, weight)

    if wts not in _compiled_cache:
        _compiled_cache[wts] = _build_program(wts)
    nc = _compiled_cache[wts]

    res = run_bass_kernel_spmd(nc, in_maps, list(range(N_CORES)))

    out_full = np.zeros((N_NODES, F), np.float32)
    for k in range(N_CORES):
        fb, fr, fm = metas[k]
        # fragment (b, m): partitions [32*(b%GPI), +32), column toff[b//GPI]+m
        dv = (
            np.asarray(res.results[k]["frag"])
            .reshape(3, F, int(toff[-1]))
            .astype(np.float32)
        )
        fvals = dv[fb % GPI, :, toff[fb // GPI] + fm]  # [n_frag, F]
        order = np.argsort(fr, kind="stable")
        sv = fvals[order]
        sr = fr[order]
        seg = np.r_[True, sr[1:] != sr[:-1]]
        segstarts = np.flatnonzero(seg)
        out_full[k * RPC + sr[segstarts]] = np.add.reduceat(
            sv, segstarts, axis=0
        )
    return out_full
